# revision 1
# baseline (speedup 1.0000x reference)
"""Trainium2 Bass kernel for local sparse attention (k=16 neighbors).

Reference computation (b=4, n=8192, k=16, d=128):
    Q = src @ Wq.T ; K = tgt @ Wk.T ; V = tgt @ Wv.T
    scores = einsum('bnkd,bnd->bnk', K, Q) / sqrt(d)
    out = einsum('bnk,bnkd->bnd', softmax(scores), V)

Algebraic restructuring (key to reaching the memory roofline):
    scores[n,k] = tgt[n,k,:] . (src[n,:] @ (Wq.T @ Wk) / sqrt(d))
    out[n,:]    = (sum_k attn[n,k] * tgt[n,k,:]) @ Wv.T
so the 34-GFLOP K/V projections are never materialized; tgt streams from
HBM exactly once in its natural layout (8KB contiguous per point).

Per 128-point tile, both heavy steps run as ONE fused DVE pass each via a
custom multiply-cumsum op (out = cumsum(in0*in1) along the free dim);
per-neighbor segment sums are recovered as differences of the cumulative
sum at segment ends.

Sharding: data-parallel over flattened (b*n) = 32768 points across 8
NeuronCores; attention is fully local per point.
"""

import math

import numpy as np

# ---- problem constants (hardcoded per the contract) ----
B, N, KNBR, D = 4, 8192, 16, 128
NCORES = 8
PTS_TOTAL = B * N               # 32768
PTS_CORE = PTS_TOTAL // NCORES  # 4096
TILE_PTS = 128
_cached = {}


def _register_mul_cumsum():
    """Register the custom DVE op out[p,t] = cumsum_t(in0[p,t]*in1[p,t])."""
    import concourse.dve_ops as dve_ops
    for op in dve_ops.OPS:
        if op.name == "MUL_CUMSUM_ANT":
            return op
    from concourse.dve_spec import Spec, Src0, Src1, AluOp, scan, lower
    from concourse.dve_uop import DveOpSpec
    from concourse.dve_table_gen import dve_ver_for
    from concourse.dve_ops import DveOp, _CUSTOM_DVE_ROW_BASE

    spec = Spec(
        body=scan(AluOp.ADD, Src0 * Src1),
        reference=lambda in0, in1, s0, s1, imm2: np.cumsum(
            in0.reshape(in0.shape[0], -1).astype(np.float32)
            * in1.reshape(in0.shape[0], -1).astype(np.float32), axis=1),
    )
    ver = dve_ver_for("TRN2")
    row = _CUSTOM_DVE_ROW_BASE + len(dve_ops.OPS)
    sha = DveOpSpec(name="MUL_CUMSUM_ANT", opcode=row,
                    uops=lower(spec, ver=ver), rd1_en=True).sha(ver)
    op = DveOp("MUL_CUMSUM_ANT", spec, subdim=False, uops_sha={ver: sha})
    dve_ops.OPS.append(op)
    dve_ops._SUB_OPCODE_FOR_NAME[op.name] = row
    dve_ops.CUSTOM_DVE_SPECS[op.name] = spec
    return op


def _build_program(pts_core=PTS_CORE, num_devices=NCORES):
    import concourse.bacc as bacc
    import concourse.bass as bass
    import concourse.tile as tile
    from concourse import mybir

    mcs = _register_mul_cumsum()
    ntiles = pts_core // TILE_PTS

    nc = bacc.Bacc("TRN2", target_bir_lowering=False, debug=False,
                   num_devices=num_devices)

    f32 = mybir.dt.float32
    src_h = nc.dram_tensor("src_sh", [pts_core, D], f32, kind="ExternalInput").ap()
    tgt_h = nc.dram_tensor("tgt_sh", [pts_core * KNBR, D], f32, kind="ExternalInput").ap()
    wqk_h = nc.dram_tensor("wqk", [D, D], f32, kind="ExternalInput").ap()
    wvt_h = nc.dram_tensor("wvt", [D, D], f32, kind="ExternalInput").ap()
    iden_h = nc.dram_tensor("iden", [D, D], f32, kind="ExternalInput").ap()
    out_h = nc.dram_tensor("out_sh", [pts_core, D], f32, kind="ExternalOutput").ap()

    ALU = mybir.AluOpType
    ACTF = mybir.ActivationFunctionType

    with tile.TileContext(nc) as tc:
        with (
            tc.tile_pool(name="consts", bufs=1) as consts,
            tc.tile_pool(name="srcp", bufs=1) as srcp,
            tc.tile_pool(name="qwp", bufs=ntiles) as qwp,
            tc.tile_pool(name="tnp", bufs=3) as tnp,
            tc.tile_pool(name="big", bufs=2) as big,
            tc.tile_pool(name="small", bufs=4) as small,
            tc.tile_pool(name="ps", bufs=4, space="PSUM") as ps,
        ):
            tgt_v = tgt_h.rearrange("(n k) d -> n k d", k=KNBR)
            src_v = src_h.rearrange("(t p) d -> p t d", p=TILE_PTS)

            # the very first DMA triggers on the (serial) Sync queue are the
            # first pairs' tgt tiles — everything downstream waits on them
            npairs = ntiles // 2
            tn_tiles = {}
            tnp_ref = tnp

            def load_tn(tp):
                p0 = tp * 2 * TILE_PTS
                tn = tnp_ref.tile([TILE_PTS, 2, KNBR, D], f32, tag="tn")
                nc.sync.dma_start(out=tn[:, 0], in_=tgt_v[p0:p0 + TILE_PTS])
                nc.sync.dma_start(out=tn[:, 1],
                                  in_=tgt_v[p0 + TILE_PTS:p0 + 2 * TILE_PTS])
                tn_tiles[tp] = tn

            for tp in range(min(2, npairs)):
                load_tn(tp)

            wqk_sb = consts.tile([D, D], f32)
            nc.sync.dma_start(out=wqk_sb, in_=wqk_h)
            wvt_sb = consts.tile([D, D], f32)
            nc.sync.dma_start(out=wvt_sb, in_=wvt_h)
            iden_sb = consts.tile([D, D], f32)
            nc.sync.dma_start(out=iden_sb, in_=iden_h)

            # queries: Qw[t] = src_tile[t] @ Wqk (Wqk includes the 1/sqrt(d)
            # scale); emitted with a small lookahead so the ACT copies
            # interleave with the main loop's exps instead of queueing ahead
            # of them (engines dispatch roughly in program order).
            src_all = srcp.tile([TILE_PTS, ntiles, D], f32)
            for c in range(0, ntiles, 4):
                ce = min(c + 4, ntiles)
                nc.sync.dma_start(out=src_all[:, c:ce, :], in_=src_v[:, c:ce, :])
            qw_tiles = {}

            def emit_qw(t):
                st_ps = ps.tile([D, TILE_PTS], f32, tag="pss")
                nc.tensor.transpose(st_ps, src_all[:, t, :], iden_sb)
                st_sb = small.tile([D, TILE_PTS], f32, tag="st")
                nc.scalar.copy(st_sb, st_ps)
                qw_ps = ps.tile([TILE_PTS, D], f32, tag="pss")
                nc.tensor.matmul(qw_ps, lhsT=st_sb, rhs=wqk_sb, start=True, stop=True)
                qw_sb = qwp.tile([TILE_PTS, D], f32, tag="qw")
                nc.scalar.copy(qw_sb, qw_ps)
                qw_tiles[t] = qw_sb

            # main loop processes PAIRS of 128-pt tiles so the small DVE ops
            # (segment diffs, reciprocal) amortize their fixed overhead
            LOOKAHEAD = 3  # pairs
            for t in range(min(2 * LOOKAHEAD, ntiles)):
                emit_qw(t)
            CW = 1 + KNBR * D          # guarded cumsum width per half
            for tp in range(npairs):
                for t in (2 * (tp + LOOKAHEAD), 2 * (tp + LOOKAHEAD) + 1):
                    if t < ntiles:
                        emit_qw(t)
                if tp + 2 < npairs:
                    load_tn(tp + 2)
                p0 = tp * 2 * TILE_PTS
                tn = tn_tiles.pop(tp)

                # ---- scores: one fused multiply-cumsum over (k,d) per half;
                # a zeroed guard column at flat offset 0 makes the
                # segment-difference a single tensor_tensor subtract.
                cum1 = big.tile([TILE_PTS, 2, CW], f32, tag="cum1")
                nc.gpsimd.memset(cum1[:, :, 0:1], 0.0)
                for h in range(2):
                    qw_sb = qw_tiles[2 * tp + h]
                    qw_bk = bass.AP(tensor=qw_sb.tensor, offset=qw_sb.offset,
                                    ap=[qw_sb.ap[0], [0, KNBR], [1, D]])
                    nc.vector._custom_dve(mcs, out=cum1[:, h, 1:],
                                          in0=tn[:, h], in1=qw_bk)
                # segment ends at flat offsets {0, 128, ..., 2048} per half
                ends1_hi = bass.AP(tensor=cum1.tensor, offset=cum1.offset + D,
                                   ap=[cum1.ap[0], [CW, 2], [D, KNBR]])
                ends1_lo = bass.AP(tensor=cum1.tensor, offset=cum1.offset,
                                   ap=[cum1.ap[0], [CW, 2], [D, KNBR]])
                scores = small.tile([TILE_PTS, 2, KNBR], f32, tag="sc")
                nc.vector.tensor_sub(scores, ends1_hi, ends1_lo)

                # ---- softmax over k (scores bounded; skip max-subtraction);
                # exp's accum_out gives the denominator in the same op
                e_sb = small.tile([TILE_PTS, 2, KNBR], f32, tag="e")
                den = small.tile([TILE_PTS, 2], f32, tag="den")
                for h in range(2):
                    nc.scalar.activation(e_sb[:, h], scores[:, h], ACTF.Exp,
                                         accum_out=den[:, h:h + 1])
                rden = small.tile([TILE_PTS, 2], f32, tag="rden")
                nc.vector.reciprocal(rden, den)

                # ---- ctx: fused multiply-cumsum over (d,k) per half: tn read
                # d-outer/k-inner; E broadcast over d
                cum2 = big.tile([TILE_PTS, 2, CW], f32, tag="cum2")
                nc.gpsimd.memset(cum2[:, :, 0:1], 0.0)
                for h in range(2):
                    tn_dk = bass.AP(tensor=tn.tensor,
                                    offset=tn.offset + h * KNBR * D,
                                    ap=[tn.ap[0], [1, D], [D, KNBR]])
                    e_bd = bass.AP(tensor=e_sb.tensor,
                                   offset=e_sb.offset + h * KNBR,
                                   ap=[e_sb.ap[0], [0, D], [1, KNBR]])
                    nc.vector._custom_dve(mcs, out=cum2[:, h, 1:],
                                          in0=e_bd, in1=tn_dk)
                # segment ends at flat offsets {0, 16, ..., 2048} per half
                ends2_hi = bass.AP(tensor=cum2.tensor, offset=cum2.offset + KNBR,
                                   ap=[cum2.ap[0], [CW, 2], [KNBR, D]])
                ends2_lo = bass.AP(tensor=cum2.tensor, offset=cum2.offset,
                                   ap=[cum2.ap[0], [CW, 2], [KNBR, D]])
                ctx = small.tile([TILE_PTS, 2, D], f32, tag="ctx")
                nc.vector.tensor_sub(ctx, ends2_hi, ends2_lo)

                # ---- out = (ctx/den) @ Wv.T, per half
                for h in range(2):
                    ctxt_ps = ps.tile([D, TILE_PTS], f32, tag="pss")
                    nc.tensor.transpose(ctxt_ps, ctx[:, h], iden_sb)
                    ctxt_sb = small.tile([D, TILE_PTS], f32, tag="ctxt")
                    nc.scalar.copy(ctxt_sb, ctxt_ps)
                    out_ps = ps.tile([TILE_PTS, D], f32, tag="pss")
                    nc.tensor.matmul(out_ps, lhsT=ctxt_sb, rhs=wvt_sb,
                                     start=True, stop=True)
                    out_sb = small.tile([TILE_PTS, D], f32, tag="outsb")
                    nc.scalar.activation(out_sb, out_ps, ACTF.Copy,
                                         scale=rden[:, h:h + 1])
                    q0 = p0 + h * TILE_PTS
                    nc.sync.dma_start(out=out_h[q0:q0 + TILE_PTS], in_=out_sb)

    nc.compile()
    return nc


def kernel(src, tgt, Wq, Wk, Wv):
    from concourse.bass_utils import run_bass_kernel_spmd

    src = np.ascontiguousarray(src, dtype=np.float32)
    tgt = np.ascontiguousarray(tgt, dtype=np.float32)

    scale = 1.0 / math.sqrt(D)
    wqk = (Wq.astype(np.float64).T @ Wk.astype(np.float64) * scale).astype(np.float32)
    wvt = np.ascontiguousarray(Wv.astype(np.float32).T)
    iden = np.eye(D, dtype=np.float32)

    src_f = src.reshape(PTS_TOTAL, D)
    tgt_f = tgt.reshape(PTS_TOTAL * KNBR, D)

    if "nc" not in _cached:
        _cached["nc"] = _build_program()
    nc = _cached["nc"]

    in_maps = []
    for c in range(NCORES):
        p0, p1 = c * PTS_CORE, (c + 1) * PTS_CORE
        in_maps.append({
            "src_sh": np.ascontiguousarray(src_f[p0:p1]),
            "tgt_sh": np.ascontiguousarray(tgt_f[p0 * KNBR:p1 * KNBR]),
            "wqk": wqk,
            "wvt": wvt,
            "iden": iden,
        })

    _cached["in_maps"] = in_maps
    res = run_bass_kernel_spmd(nc, in_maps, core_ids=list(range(NCORES)))
    out = np.concatenate([r["out_sh"] for r in res.results], axis=0)
    return out.reshape(B, N, D).astype(np.float32)


def __getattr__(name):
    if name == "_last_in_maps":
        return _cached.get("in_maps")
    raise AttributeError(name)



# revision 2
# speedup vs baseline: 1.6615x; 1.6615x over previous
"""Trainium2 Bass kernel for local sparse attention (k=16 neighbors).

Reference computation (b=4, n=8192, k=16, d=128):
    Q = src @ Wq.T ; K = tgt @ Wk.T ; V = tgt @ Wv.T
    scores = einsum('bnkd,bnd->bnk', K, Q) / sqrt(d)
    out = einsum('bnk,bnkd->bnd', softmax(scores), V)

Restructured so the 34-GFLOP K/V projections are never materialized:
    scores[n,k] = tgt[n,k,:] . qw[n,:],  qw = src @ (Wq.T Wk / sqrt(d))
    out[n,:]    = (sum_k e[n,k] * tgt[n,k,:]) @ Wv.T / sum_k e[n,k]

Device pipeline per 128-point tile (all tensors fp16, fp32 accumulation):
  1. SEGDOT_ANT   (custom DVE op, 2x_1p fp16 mode, hand-written uops):
     per-page dot products via a segmented multiply-cumsum whose
     accumulator RESETS at each 128-element page boundary; page-end
     positions hold scores[p, k]. Two fp16 MACs per lane-cycle.
  2. ScalarE exp on the 16 page-end values (read twice each ->
     duplicated pairs e2[p,k,2]), accum_out = 2*sum_k e.
  3. PAGESCALE_ANT (custom DVE op, 2x_1p): scaled[p,k,:] = tn[p,k,:] *
     e[p,k], with e held in swap flops and re-latched per page (src1 is
     consumed as one 4-byte pair per page -> e duplicated pairs).
  4. TensorE: 16 accumulating identity-matmuls produce
     ctxT[d, p] = sum_k scaled[p,k,d] directly in PSUM (transpose and
     reduction in one), then out_raw = ctxT.T @ Wv.T via one more matmul.
  5. fp16 out_raw + fp32 den DMA'd out; host divides by den.

Host does the tiny O(1/32 of flops) parts: qw projection, fp16 casts,
final divide. Sharding: data-parallel over (b*n) across 8 cores.
"""

import math

import numpy as np

B, N_SEQ, KNBR, D = 4, 8192, 16, 128
NCORES = 8
PTS_TOTAL = B * N_SEQ            # 32768
PTS_CORE = PTS_TOTAL // NCORES   # 4096
TILE = 128
NTILES = PTS_CORE // TILE        # 32
_cached = {}

# --------------------------------------------------------------------------
# Custom DVE ops (hand-written uop programs, 1x + 2x_1p variants)
# --------------------------------------------------------------------------


def _register_dve_ops():
    from concourse.dve_ops import (
        DveOp, OPS, CUSTOM_DVE_SPECS, _SUB_OPCODE_FOR_NAME,
        _CUSTOM_DVE_ROW_BASE,
    )
    have = {op.name: op for op in OPS}
    if "SEGDOT_ANT" in have:
        return have["SEGDOT_ANT"], have["PAGESCALE_ANT"]

    from dataclasses import dataclass
    from concourse.dve_spec import (
        Spec, Src0, Src1, C3, scan, AluOp, _spill_c3_to_src1,
    )
    from concourse.dve_uop import (
        DveOpSpec, UopConfig, AluInp, InpSel, OutSel, OutPath,
        Trigger, DelayInp, ENABLE,
    )

    Dl = [AluInp.PREV_DELAY_0, AluInp.PREV_DELAY_1, AluInp.PREV_DELAY_2,
          AluInp.PREV_DELAY_3, AluInp.PREV_DELAY_4, AluInp.PREV_DELAY_5]
    PREV, CURR, SWAP = (AluInp.PREV_ALU_OUT, AluInp.CURR_ALU_OUT,
                        AluInp.CURR_SWAP_OUT)
    T_STEADY = (Trigger.SRC_TENSOR_DONE, Trigger.SUB_DIM_DONE, Trigger.NONE)
    T_STEP = (Trigger.SRC_TENSOR_DONE, Trigger.SUB_DIM_DONE, Trigger.COUNT)
    T_SEED = (Trigger.COUNT, Trigger.NONE, Trigger.NONE)

    def mk(inputs, lanes):
        u = UopConfig()
        for lane_idx, sel in inputs:
            u.enable_input(sel, lane_idx + 1)
        for b in range(8):
            u.datapath_config[b].pass_through_alu()
            u.datapath_config[b].pass_through_delay(*lanes)
        return u

    def seed(u):
        u.trigger, u.repeat_count, u.next_uop = T_SEED, 1, (1, 0, 0)
        return u

    def steady(u, write=True):
        u.require_inp0 = u.require_inp1 = 1
        if write:
            u.enable_output(OutSel.ALU_OUT, OutPath.WR0_LO)
        u.trigger, u.next_uop = T_STEADY, (0, 2, 0)
        return u

    def step(u, write=True):
        u.require_inp0 = u.require_inp1 = 1
        if write:
            u.enable_output(OutSel.ALU_OUT, OutPath.WR0_LO)
        u.trigger, u.next_uop, u.repeat_count = T_STEP, (0, 2, 1), 1
        return u

    # ---- SEGDOT: reset-cumsum of in0*in1 over pages -----------------------
    def segdot_1x():
        INP = [(0, InpSel.SRC_0), (1, InpSel.SRC_1), (2, InpSel.ZERO)]

        def base():
            u = mk(INP, (0, 1, 2))
            u.datapath_config[0].enable_alu(AluOp.MULTIPLY, Dl[0], Dl[1])
            u.datapath_config[1].enable_alu(AluOp.ADD, CURR, PREV)
            return u

        s0 = base()
        s0.datapath_config[1].enable_alu(AluOp.BYPASS, Dl[2])   # acc <- 0
        s1 = steady(base())
        s2 = base()
        s2.datapath_config[1].enable_alu(AluOp.BYPASS, PREV)    # acc <- m
        return [seed(s0), s1, step(s2)]

    def segdot_2x():
        INP = [(0, InpSel.SRC_0), (1, InpSel.SRC_1),
               (2, InpSel.SRC_0_HI), (3, InpSel.SRC_1_HI), (4, InpSel.ZERO)]

        def base():
            u = mk(INP, (0, 1, 2, 3, 4))
            dp = u.datapath_config
            dp[0].enable_alu(AluOp.MULTIPLY, Dl[0], Dl[1])           # m0
            dp[1].enable_alu(AluOp.MULTIPLY, Dl[2], Dl[3])           # m1
            dp[1].enable_delay_from_src(DelayInp.PREV_ALU_OUT, 5)    # ch5<-m0
            for b in range(2, 8):
                dp[b].pass_through_delay(5)
            dp[2].enable_alu(AluOp.ADD, PREV, Dl[5])                 # t=m0+m1
            dp[3].enable_alu(AluOp.ADD, CURR, PREV)                  # acc+=t
            return u

        s0 = base()
        s0.datapath_config[3].enable_alu(AluOp.BYPASS, Dl[4])        # acc<-0
        s1 = steady(base())
        s1.enable_output(OutSel.ALU_OUT, OutPath.WR0_HI)  # pair-cumsum both halves
        s2 = base()
        s2.datapath_config[3].enable_alu(AluOp.BYPASS, PREV)         # acc<-t
        step(s2)
        s2.enable_output(OutSel.ALU_OUT, OutPath.WR0_HI)
        return [seed(s0), s1, s2]

    # ---- PAGESCALE: in0 * e[page], e latched per page from src1 ----------
    def pagescale_1x():
        INP = [(0, InpSel.SRC_0), (1, InpSel.SRC_1)]
        init = mk(INP, (0, 1))
        init.datapath_config[0].enable_alu(AluOp.BYPASS, Dl[1])
        init.datapath_config[0].swap_enable = ENABLE
        init.require_inp1 = 1
        seed(init)

        st = mk(INP, (0, 1))
        st.datapath_config[0].enable_alu(AluOp.MULTIPLY, Dl[0], SWAP)
        st.require_inp0 = 1
        st.enable_output(OutSel.ALU_OUT, OutPath.WR0_LO)
        st.trigger, st.next_uop = T_STEADY, (0, 2, 0)

        sp = mk(INP, (0, 1))
        sp.datapath_config[0].enable_alu(AluOp.BYPASS, Dl[1])
        sp.datapath_config[0].swap_enable = ENABLE
        sp.datapath_config[1].enable_alu(AluOp.MULTIPLY, Dl[0], PREV)
        step(sp)
        return [init, st, sp]

    def pagescale_2x():
        INP = [(0, InpSel.SRC_0), (1, InpSel.SRC_0_HI), (2, InpSel.SRC_1)]
        init = mk(INP, (0, 1, 2))
        init.datapath_config[0].enable_alu(AluOp.BYPASS, Dl[2])
        init.datapath_config[0].swap_enable = ENABLE
        init.datapath_config[1].enable_alu(AluOp.BYPASS, PREV)
        init.datapath_config[1].swap_enable = ENABLE
        init.require_inp1 = 1
        seed(init)

        st = mk(INP, (0, 1, 2))
        dp = st.datapath_config
        dp[0].enable_alu(AluOp.MULTIPLY, Dl[0], SWAP)                # r0
        dp[1].enable_alu(AluOp.MULTIPLY, Dl[1], SWAP)                # r1
        dp[1].enable_delay_from_src(DelayInp.PREV_ALU_OUT, 3)        # ch3<-r0
        for b in range(2, 8):
            dp[b].pass_through_delay(3)
        st.require_inp0 = 1
        st.enable_output(OutSel.DELAY_3, OutPath.WR0_LO)             # r0 even
        st.enable_output(OutSel.ALU_OUT, OutPath.WR0_HI)             # r1 odd
        st.trigger, st.next_uop = T_STEADY, (0, 2, 0)

        sp = mk(INP, (0, 1, 2))
        dp = sp.datapath_config
        dp[0].enable_alu(AluOp.BYPASS, Dl[2])                        # e_new
        dp[0].swap_enable = ENABLE
        dp[1].enable_alu(AluOp.BYPASS, PREV)
        dp[1].swap_enable = ENABLE
        dp[2].enable_alu(AluOp.MULTIPLY, Dl[0], PREV)                # r0
        dp[2].enable_delay_from_src(DelayInp.PREV_ALU_OUT, 3)        # ch3<-e
        dp[3].enable_alu(AluOp.MULTIPLY, Dl[1], Dl[3])               # r1
        dp[3].enable_delay_from_src(DelayInp.PREV_ALU_OUT, 4)        # ch4<-r0
        for b in range(3, 8):
            dp[b].pass_through_delay(3)
        for b in range(4, 8):
            dp[b].pass_through_delay(4)
        sp.require_inp0 = sp.require_inp1 = 1
        sp.enable_output(OutSel.DELAY_4, OutPath.WR0_LO)
        sp.enable_output(OutSel.ALU_OUT, OutPath.WR0_HI)
        sp.trigger, sp.next_uop, sp.repeat_count = T_STEP, (0, 2, 1), 1
        return [init, st, sp]

    @dataclass(frozen=True)
    class HandDveOp(DveOp):
        raw_v3: "DveOpSpec | None" = None

        def compile(self, ver):
            assert ver == "v3", f"hand-built op only has v3 uops, got {ver}"
            return self.raw_v3

    def segdot_ref(in0, in1, s0, s1, imm2):
        P, N = in0.shape[0], in0.shape[-1]
        S = int(np.prod(in0.shape[1:-1]))
        a = in0.reshape(P, S, N).astype(np.float32)
        b = np.asarray(in1, np.float32)
        b = (np.broadcast_to(b.reshape(P, 1, N), a.shape)
             if b.size == P * N else b.reshape(a.shape))
        return np.cumsum(a * b, axis=-1).reshape(in0.shape)

    def pagescale_ref(in0, in1, s0, s1, imm2):
        P, N = in0.shape[0], in0.shape[-1]
        S = int(np.prod(in0.shape[1:-1]))
        a = in0.reshape(P, S, N).astype(np.float32)
        e = np.asarray(in1, np.float32).reshape(P, -1)[:, ::2][:, :S]
        return (a * e[:, :, None]).reshape(in0.shape)

    ops = []
    for name, spec, u1, u2 in (
        ("SEGDOT_ANT",
         Spec(body=scan(AluOp.ADD, Src0 * Src1), reference=segdot_ref),
         segdot_1x(), segdot_2x()),
        ("PAGESCALE_ANT",
         Spec(body=Src0 * _spill_c3_to_src1(C3), reference=pagescale_ref),
         pagescale_1x(), pagescale_2x()),
    ):
        row = _CUSTOM_DVE_ROW_BASE + len(OPS)
        raw = DveOpSpec(name=name, opcode=row, uops=u1, uops_2x=u2,
                        perf_max=1, rd1_en=True)
        raw.validate("v3")
        op = HandDveOp(name=name, spec=spec, subdim=True,
                       uops_sha={"v3": raw.sha("v3")}, raw_v3=raw)
        OPS.append(op)
        _SUB_OPCODE_FOR_NAME[name] = row
        CUSTOM_DVE_SPECS[name] = spec
        ops.append(op)
    return ops[0], ops[1]


def _emit_custom(nc, op, *, out, in0, in1, perf_max=1):
    """Like nc.vector._custom_dve but with an explicit perf_max."""
    from concourse import bass_isa, mybir
    from concourse.dve_ops import get_dve_sub_opcode

    vec = nc.vector
    m = vec.bass.m
    if op.name not in m.ant_custom_dve_ops:
        m.ant_custom_dve_ops = sorted({*m.ant_custom_dve_ops, op.name})
    opt = not op.subdim
    in1_elementwise = len(in1.shape) > 2
    shape = (bass_isa.CustomDveShape.STT if in1_elementwise
             else bass_isa.CustomDveShape.TTSS)
    isa_opcode = vec.bass.isa.Opcode[
        f"NEURON_ISA_TPB_OPCODE_CUSTOM_DVE_ANT_{shape.slot()}"
    ].value
    zero = mybir.ImmediateValue(dtype=mybir.dt.float32, value=0.0)
    ins = [vec.lower_ap(in0, for_isa=True, opt=opt),
           vec.lower_ap(in1, for_isa=True, opt=opt), zero, zero]
    outs = [vec.lower_ap(out, for_isa=True, opt=opt)]
    return vec.add_instruction(bass_isa.InstCustomDveAnt(
        name=vec.bass.get_next_instruction_name(),
        op_name=op.name, rd1_en=True, subdim=0x02, imm2=0.0,
        shape=shape, row=get_dve_sub_opcode(op.name), isa_opcode=isa_opcode,
        perf_max=perf_max, ins=ins, outs=outs))


# --------------------------------------------------------------------------
# Device program
# --------------------------------------------------------------------------


def _build_program(pts_core=PTS_CORE, num_devices=NCORES):
    import concourse.bacc as bacc
    import concourse.bass as bass
    import concourse.tile as tile
    from concourse import mybir

    SEG, PSC = _register_dve_ops()
    ntiles = pts_core // TILE

    nc = bacc.Bacc("TRN2", target_bir_lowering=False, debug=False,
                   num_devices=num_devices)

    f32, f16 = mybir.dt.float32, mybir.dt.float16
    tgt_h = nc.dram_tensor("tgt_sh", [pts_core * KNBR, D], f16,
                           kind="ExternalInput").ap()
    qw_h = nc.dram_tensor("qw_sh", [pts_core, D], f16,
                          kind="ExternalInput").ap()
    wvt_h = nc.dram_tensor("wvt", [D, D], f16, kind="ExternalInput").ap()
    iden_h = nc.dram_tensor("iden", [D, D], f16, kind="ExternalInput").ap()
    out_h = nc.dram_tensor("out_sh", [pts_core, D], f16,
                           kind="ExternalOutput").ap()
    den_h = nc.dram_tensor("den_sh", [pts_core, 1], f32,
                           kind="ExternalOutput").ap()

    ACTF = mybir.ActivationFunctionType

    with tile.TileContext(nc) as tc:
        with (
            tc.tile_pool(name="consts", bufs=1) as consts,
            tc.tile_pool(name="qwp", bufs=1) as qwp,
            tc.tile_pool(name="outp", bufs=1) as outp,
            tc.tile_pool(name="tnp", bufs=4) as tnp,
            tc.tile_pool(name="cump", bufs=3) as cump,
            tc.tile_pool(name="sclp", bufs=3) as sclp,
            tc.tile_pool(name="smal", bufs=4) as smal,
            tc.tile_pool(name="ps", bufs=4, space="PSUM") as ps,
        ):
            tgt_v = tgt_h.rearrange("(n k) d -> n k d", k=KNBR)
            qw_v = qw_h.rearrange("(t p) d -> p t d", p=TILE)
            out_v = out_h.rearrange("(t p) d -> p t d", p=TILE)
            den_v = den_h.rearrange("(t p) one -> p t one", p=TILE)

            # tgt tile prefetch (4KB contiguous per point-row)
            tn_tiles = {}

            def load_tn(t):
                tn = tnp.tile([TILE, KNBR, D], f16, tag="tn")
                p0 = t * TILE
                nc.sync.dma_start(out=tn, in_=tgt_v[p0:p0 + TILE])
                tn_tiles[t] = tn

            for t in range(min(3, ntiles)):
                load_tn(t)

            wvt_sb = consts.tile([D, D], f16)
            nc.sync.dma_start(out=wvt_sb, in_=wvt_h)
            iden_sb = consts.tile([D, D], f16)
            nc.sync.dma_start(out=iden_sb, in_=iden_h)

            qw_all = qwp.tile([TILE, ntiles, D], f16)
            for c in range(0, ntiles, 8):
                ce = min(c + 8, ntiles)
                nc.sync.dma_start(out=qw_all[:, c:ce, :], in_=qw_v[:, c:ce, :])

            out_all = outp.tile([TILE, ntiles, D], f16)
            den_all = outp.tile([TILE, ntiles, 1], f32)

            OGRP = 8  # output DMA granularity (tiles)
            for t in range(ntiles):
                if t + 3 < ntiles:
                    load_tn(t + 3)
                tn = tn_tiles.pop(t)

                # scores: segmented dot products, page ends hold the result
                cum = cump.tile([TILE, KNBR, D], f16, tag="cum")
                qw_bk = bass.AP(tensor=qw_all.tensor,
                                offset=qw_all.offset + t * D,
                                ap=[qw_all.ap[0], [0, KNBR], [1, D]])
                _emit_custom(nc, SEG, out=cum, in0=tn, in1=qw_bk)

                # e2[p, k, 2] = exp(score[p, k]) twice; accum = 2*sum_k e
                ends = bass.AP(tensor=cum.tensor, offset=cum.offset + (D - 1),
                               ap=[cum.ap[0], [D, KNBR], [0, 2]])
                e2 = smal.tile([TILE, KNBR, 2], f16, tag="e2")
                nc.scalar.activation(e2, ends, ACTF.Exp,
                                     accum_out=den_all[:, t, :])

                # scaled[p,k,:] = tn[p,k,:] * e[p,k]
                scaled = sclp.tile([TILE, KNBR, D], f16, tag="scl")
                _emit_custom(nc, PSC, out=scaled, in0=tn,
                             in1=e2.rearrange("p k two -> p (k two)"))

                # ctxT[d, p] = sum_k scaled[p, k, d] via accumulating
                # identity-matmuls (transpose + reduce in one)
                ps_ctxT = ps.tile([D, TILE], f32, tag="psc")
                for k in range(KNBR):
                    nc.tensor.matmul(ps_ctxT, lhsT=scaled[:, k, :],
                                     rhs=iden_sb, start=(k == 0),
                                     stop=(k == KNBR - 1))
                ctxT_sb = smal.tile([D, TILE], f16, tag="ctxT")
                nc.scalar.copy(ctxT_sb, ps_ctxT)

                # out_raw = ctx @ Wv.T
                ps_out = ps.tile([TILE, D], f32, tag="pso")
                nc.tensor.matmul(ps_out, lhsT=ctxT_sb, rhs=wvt_sb,
                                 start=True, stop=True)
                nc.scalar.copy(out_all[:, t, :], ps_out)

                if (t + 1) % OGRP == 0:
                    t0 = t + 1 - OGRP
                    nc.sync.dma_start(out=out_v[:, t0:t + 1, :],
                                      in_=out_all[:, t0:t + 1, :])
            nc.sync.dma_start(out=den_v, in_=den_all)

    nc.compile()
    return nc


# --------------------------------------------------------------------------
# Host wrapper
# --------------------------------------------------------------------------


def kernel(src, tgt, Wq, Wk, Wv):
    from concourse.bass_utils import run_bass_kernel_spmd

    scale = 1.0 / math.sqrt(D)
    wqk = (Wq.astype(np.float64).T @ Wk.astype(np.float64)
           * scale).astype(np.float32)
    qw = (np.ascontiguousarray(src, dtype=np.float32).reshape(PTS_TOTAL, D)
          @ wqk).astype(np.float16)
    tn = np.ascontiguousarray(tgt, dtype=np.float32) \
        .reshape(PTS_TOTAL * KNBR, D).astype(np.float16)
    wvt = np.ascontiguousarray(Wv.astype(np.float32).T).astype(np.float16)
    iden = np.eye(D, dtype=np.float16)

    if "nc" not in _cached:
        _cached["nc"] = _build_program()
    nc = _cached["nc"]

    in_maps = []
    for c in range(NCORES):
        p0, p1 = c * PTS_CORE, (c + 1) * PTS_CORE
        in_maps.append({
            "tgt_sh": tn[p0 * KNBR:p1 * KNBR],
            "qw_sh": qw[p0:p1],
            "wvt": wvt,
            "iden": iden,
        })

    _cached["in_maps"] = in_maps
    res = run_bass_kernel_spmd(nc, in_maps, core_ids=list(range(NCORES)))
    out_raw = np.concatenate(
        [r["out_sh"].astype(np.float32) for r in res.results], axis=0)
    den = np.concatenate(
        [r["den_sh"].astype(np.float32) for r in res.results], axis=0) * 0.5
    out = out_raw / den
    return out.reshape(B, N_SEQ, D).astype(np.float32)


def __getattr__(name):
    if name == "_last_in_maps":
        return _cached.get("in_maps")
    raise AttributeError(name)
